# revision 21
# baseline (speedup 1.0000x reference)
"""Trainium2 Bass kernel for nn_MiniBrain (2-layer binarized-weight spiking MLP).

Computes spk2 = ((x @ sign(W1).T > 1) @ sign(W2).T > 1).astype(f32)
for x [8192, 4096], W1/W2 [4096, 4096], data-parallel over batch on 8 cores.

Layer-1 numerics (1.5-pass scheme):
  - Main pass: xm = fp16(x) moving x fp8 sign(W1) stationary; products exact
    on the PE (fp16 x {-1,+1}), fp32 PSUM accumulation.
  - Residual pass: r = x - fp16(x) captured as rq = e4m3(r * 512), multiplied
    by weights sign(W1) * 2^-9 (exact e4m3 subnormal) in fp8 DoubleRow mode,
    accumulating rq * sign * 2^-9 = r-hat * sign into the SAME PSUM bank.
    Combined representation error ~2^-15 of |x| per term -> ~2.3k output
    flips (sim), rel err ~1.2e-2, inside the 2e-2 gate.
  - Spike threshold: tensor_scalar is_gt 1.0 produces exact 0.0/1.0.
  - Layer 2: spikes {0,1} and sign(W2) {-1,+1} exact in fp8e4 DoubleRow;
    all partial sums small integers, exact in fp32: bit-exact given spk1.

Cost: per core 4096 matmuls x 216ns (vs 5120 for the 2xfp16 baseline).
"""
import numpy as np
import ml_dtypes

B = 8192
D = 4096          # NUM_INPUTS == NUM_HIDDEN == NUM_OUTPUTS
NCORES = 8
BC = B // NCORES  # batch rows per core (1024)
P = 128
NIO = D // P      # 32 contraction chunks
NHT = D // P      # 32 hidden tiles
BBLK = 512        # batch block per core
NBLK = BC // BBLK # 2 blocks
NBT = BBLK // P   # 4 L2 batch tiles per block
OGS = 512         # L2 output-column group size
NOG = D // OGS    # 8
VTH = 1.0
RSH = 9           # residual weight scale 2^-RSH (e4m3 subnormal, exact)
RSCALE = float(2 ** RSH)

F8 = ml_dtypes.float8_e4m3

_cache = {}


def _build_program():
    import concourse.bacc as bacc
    import concourse.mybir as mybir
    from concourse.tile import TileContext

    nc = bacc.Bacc("TRN2", target_bir_lowering=False, debug=False)
    dt = mybir.dt

    # Inputs (host-pretiled layouts; see kernel() below).
    xm = nc.declare_dram_parameter("xm", [P, NIO, BC], dt.float16, isOutput=False)
    rq = nc.declare_dram_parameter("rq", [P, NIO, BC], dt.float8e4, isOutput=False)
    # w1[ht, p(i), io, h] = sign(W1)[ht*128+h, io*128+p]
    w1 = nc.declare_dram_parameter("w1", [NHT, P, NIO, P], dt.float8e4, isOutput=False)
    # w2[og, p(h_inner), hc, oo] = sign(W2).T[hc*128+p, og*OGS+oo]
    w2 = nc.declare_dram_parameter("w2", [NOG, P, NIO, OGS], dt.float8e4,
                                   isOutput=False)
    out = nc.declare_dram_parameter("out", [BC, D], dt.float8e4, isOutput=True)

    # x DMA chunk sizes in io units (first chunks small so the PE starts fast)
    XCHS = [2, 2, 4, 4, 4, 4, 4, 4, 4]
    IO2C = []  # io -> (chunk idx, offset)
    for ci, sz in enumerate(XCHS):
        for off in range(sz):
            IO2C.append((ci, off))

    with TileContext(nc) as tc:
        with tc.tile_pool(name="xpool", bufs=1) as xpool, \
             tc.tile_pool(name="wpool", bufs=5) as wpool, \
             tc.tile_pool(name="wrpool", bufs=4) as wrpool, \
             tc.tile_pool(name="w2pool", bufs=2) as w2pool, \
             tc.tile_pool(name="spool", bufs=2) as spool, \
             tc.tile_pool(name="opool", bufs=8) as opool, \
             tc.tile_pool(name="ps1", bufs=4, space="PSUM") as ps1, \
             tc.tile_pool(name="ps2", bufs=2, space="PSUM") as ps2:
            for blk in range(NBLK):
                bsl = slice(blk * BBLK, (blk + 1) * BBLK)
                # First group's weights before the bulk x stream so the PE
                # starts as soon as the first x chunk lands.
                w1ts = {0: wpool.tile([P, NIO, P], dt.float8e4,
                                      name=f"w1t_{blk}_0", tag="w1t")}
                nc.sync.dma_start(w1ts[0], w1[0])
                # x main/residual, chunked so matmuls pace the DMA stream.
                # All xm chunks first (consumed first), then rq chunks.
                xmc = []
                rqc = []
                io0 = 0
                for ci, sz in enumerate(XCHS):
                    t = xpool.tile([P, sz, BBLK], dt.float16,
                                   name=f"xm_{blk}_{ci}", tag=f"xm_{ci}")
                    nc.scalar.dma_start(t, xm[:, io0:io0 + sz, bsl])
                    xmc.append(t)
                    io0 += sz
                io0 = 0
                for ci, sz in enumerate(XCHS):
                    t = xpool.tile([P, sz, BBLK], dt.float8e4,
                                   name=f"rq_{blk}_{ci}", tag=f"rq_{ci}")
                    nc.scalar.dma_start(t, rq[:, io0:io0 + sz, bsl])
                    rqc.append(t)
                    io0 += sz

                # Layer 1: spk1[p(h_inner), ht, b] for this block
                spk1 = spool.tile([P, NHT, BBLK], dt.float8e4, name=f"spk1_{blk}",
                                  tag="spk1")
                # Groups of GB hidden tiles share one Normal->DoubleRow mode
                # switch: all GB main passes (fp16), then all GB residual
                # passes (DR). Mode switches block the LDWEIGHTS pull-ahead
                # (~230ns each), so fewer switches = fewer PE stalls.
                GB = 4
                for gb in range(NHT // GB):
                    hts = list(range(gb * GB, (gb + 1) * GB))
                    psums = {}
                    w1rts = {}
                    for ht in hts:
                        if ht not in w1ts:
                            w1ts[ht] = wpool.tile(
                                [P, NIO, P], dt.float8e4,
                                name=f"w1t_{blk}_{ht}", tag="w1t")
                            nc.sync.dma_start(w1ts[ht], w1[ht])
                        # residual weights derived on-chip: w1r = w1 * 2^-RSH
                        # (exact e4m3 subnormal); saves a second weight stream.
                        w1rts[ht] = wrpool.tile([P, NIO, P], dt.float8e4,
                                                name=f"w1rt_{blk}_{ht}",
                                                tag="w1rt")
                        nc.gpsimd.tensor_scalar(
                            w1rts[ht], w1ts[ht], 2.0 ** -RSH, None,
                            mybir.AluOpType.mult
                        )
                        psums[ht] = ps1.tile([P, BBLK], dt.float32,
                                             name=f"ps1_{blk}_{ht}", tag="ps1")
                    for ht in hts:
                        w1t = w1ts[ht]
                        for io in range(NIO):
                            ci, off = IO2C[io]
                            nc.tensor.matmul(
                                psums[ht], w1t[:, io, :], xmc[ci][:, off, :],
                                start=(io == 0), stop=False,
                            )
                    for ht in hts:
                        w1rt = w1rts[ht]
                        for j in range(NIO // 2):
                            ci, off = IO2C[2 * j]
                            nc.tensor.matmul(
                                psums[ht], w1rt[:, 2 * j:2 * j + 2, :],
                                rqc[ci][:, off:off + 2, :],
                                start=False, stop=(j == NIO // 2 - 1),
                                perf_mode=mybir.MatmulPerfMode.DoubleRow,
                            )
                        nc.vector.tensor_scalar(
                            spk1[:, ht, :], psums[ht], VTH, None,
                            mybir.AluOpType.is_gt
                        )
                        w1ts.pop(ht)

                # Layer 2: out[b, o] for this block (fp8 DoubleRow: hc pairs).
                # og pairs share each spk1 stationary across 2 consecutive
                # matmuls so the 256-col DoubleRow LDWEIGHTS can be deduped /
                # hidden under the other stream.
                for ogp in range(NOG // 2):
                    w2ts = []
                    for half in range(2):
                        og = 2 * ogp + half
                        w2t = w2pool.tile([P, NIO, OGS], dt.float8e4,
                                          name=f"w2t_{blk}_{og}", tag=f"w2t{half}")
                        nc.scalar.dma_start(w2t, w2[og])
                        w2ts.append(w2t)
                    for bt in range(NBT):
                        b0 = bt * P
                        psums = [
                            ps2.tile([P, OGS], dt.float32,
                                     name=f"ps2_{blk}_{2 * ogp + half}_{bt}",
                                     tag=f"ps2{half}")
                            for half in range(2)
                        ]
                        for j in range(NIO // 2):
                            lhsT = spk1[:, 2 * j:2 * j + 2, b0:b0 + P]
                            for half in range(2):
                                nc.tensor.matmul(
                                    psums[half],
                                    lhsT,
                                    w2ts[half][:, 2 * j:2 * j + 2, :],
                                    start=(j == 0), stop=(j == NIO // 2 - 1),
                                    perf_mode=mybir.MatmulPerfMode.DoubleRow,
                                )
                        for half in range(2):
                            og = 2 * ogp + half
                            o0 = og * OGS
                            ot = opool.tile([P, OGS], dt.float8e4,
                                            name=f"ot_{blk}_{og}_{bt}", tag="ot")
                            nc.vector.tensor_scalar(
                                ot, psums[half], VTH, None, mybir.AluOpType.is_gt
                            )
                            nc.sync.dma_start(
                                out[blk * BBLK + b0: blk * BBLK + b0 + P,
                                    o0:o0 + OGS], ot
                            )

    nc.finalize()
    return nc


def _get_program():
    if "nc" not in _cache:
        _cache["nc"] = _build_program()
    return _cache["nc"]


def _prep_weights(W1, W2):
    # w1[ht, p, io, h] = sign(W1)[ht*128+h, io*128+p]
    S1 = np.sign(W1).astype(np.float32)
    w1 = np.ascontiguousarray(
        S1.reshape(NHT, P, NIO, P).transpose(0, 3, 2, 1)
    ).astype(F8)
    # w2[og, p, hc, oo] = sign(W2).T[hc*128+p, og*OGS+oo]
    S2T = np.ascontiguousarray(np.sign(W2).astype(np.float32).T)
    w2 = np.ascontiguousarray(
        S2T.reshape(NIO, P, NOG, OGS).transpose(2, 1, 0, 3)
    ).astype(F8)
    return w1, w2


def _tile_x(a):
    # [BC, D] -> [p, io, b]: out[p, io, b] = a[b, io*128+p]
    return np.ascontiguousarray(a.T.reshape(NIO, P, BC).transpose(1, 0, 2))


def _split_x(xs):
    # xs: [BC, D] fp32 -> fp16 main term + e4m3 residual (x512), tiled
    xm = xs.astype(np.float16)
    r = (xs - xm.astype(np.float32)) * RSCALE
    rq = r.astype(F8)
    return _tile_x(xm), _tile_x(rq)


def kernel(x, W1, W2, layer_idx):
    from concourse.bass_utils import run_bass_kernel_spmd

    x = np.asarray(x, dtype=np.float32)
    W1 = np.asarray(W1, dtype=np.float32)
    W2 = np.asarray(W2, dtype=np.float32)

    nc = _get_program()
    w1, w2 = _prep_weights(W1, W2)

    in_maps = []
    for c in range(NCORES):
        xs = x[c * BC:(c + 1) * BC]
        xm, rq = _split_x(xs)
        in_maps.append({"xm": xm, "rq": rq, "w1": w1, "w2": w2})

    res = run_bass_kernel_spmd(nc, in_maps, list(range(NCORES)))
    outs = [res.results[c]["out"].astype(np.float32) for c in range(NCORES)]
    return np.concatenate(outs, axis=0)


# revision 22
# speedup vs baseline: 3.5332x; 3.5332x over previous
"""Trainium2 Bass kernel for nn_MiniBrain (2-layer binarized-weight spiking MLP).

Computes spk2 = ((x @ sign(W1).T > 1) @ sign(W2).T > 1).astype(f32)
for x [8192, 4096], W1/W2 [4096, 4096], data-parallel over batch on 8 cores.

Layer-1 numerics (1.5-pass scheme):
  - Main pass: xm = fp16(x) moving x fp8 sign(W1) stationary; products exact
    on the PE (fp16 x {-1,+1}), fp32 PSUM accumulation.
  - Residual pass: r = x - fp16(x) captured as rq = e4m3(r * 512), multiplied
    by weights sign(W1) * 2^-9 (exact e4m3 subnormal) in fp8 DoubleRow mode,
    accumulating rq * sign * 2^-9 = r-hat * sign into the SAME PSUM bank.
    Combined representation error ~2^-15 of |x| per term -> ~2.3k output
    flips (sim), rel err ~1.2e-2, inside the 2e-2 gate.
  - Spike threshold: tensor_scalar is_gt 1.0 produces exact 0.0/1.0.
  - Layer 2: spikes {0,1} and sign(W2) {-1,+1} exact in fp8e4 DoubleRow;
    all partial sums small integers, exact in fp32: bit-exact given spk1.

Cost: per core 4096 matmuls x 216ns (vs 5120 for the 2xfp16 baseline).
"""
import numpy as np
import ml_dtypes

B = 8192
D = 4096          # NUM_INPUTS == NUM_HIDDEN == NUM_OUTPUTS
NCORES = 8
BC = B // NCORES  # batch rows per core (1024)
P = 128
NIO = D // P      # 32 contraction chunks
NHT = D // P      # 32 hidden tiles
BBLK = 512        # batch block per core
NBLK = BC // BBLK # 2 blocks
NBT = BBLK // P   # 4 L2 batch tiles per block
OGS = 512         # L2 output-column group size
NOG = D // OGS    # 8
VTH = 1.0
RSH = 9           # residual weight scale 2^-RSH (e4m3 subnormal, exact)
RSCALE = float(2 ** RSH)

F8 = ml_dtypes.float8_e4m3

_cache = {}


def _build_program():
    import concourse.bacc as bacc
    import concourse.mybir as mybir
    from concourse.tile import TileContext

    nc = bacc.Bacc("TRN2", target_bir_lowering=False, debug=False)
    dt = mybir.dt

    # Inputs (host-pretiled layouts; see kernel() below).
    xm = nc.declare_dram_parameter("xm", [P, NIO, BC], dt.float16, isOutput=False)
    rq = nc.declare_dram_parameter("rq", [P, NIO, BC], dt.float8e4, isOutput=False)
    # w1[ht, p(i), io, h] = sign(W1)[ht*128+h, io*128+p]
    w1 = nc.declare_dram_parameter("w1", [NHT, P, NIO, P], dt.float8e4, isOutput=False)
    # w2[og, p(h_inner), hc, oo] = sign(W2).T[hc*128+p, og*OGS+oo]
    w2 = nc.declare_dram_parameter("w2", [NOG, P, NIO, OGS], dt.float8e4,
                                   isOutput=False)
    out = nc.declare_dram_parameter("out", [BC, D], dt.float8e4, isOutput=True)

    # x DMA chunk sizes in io units (first chunks small so the PE starts fast)
    XCHS = [2, 2, 4, 4, 4, 4, 4, 4, 4]
    IO2C = []  # io -> (chunk idx, offset)
    for ci, sz in enumerate(XCHS):
        for off in range(sz):
            IO2C.append((ci, off))

    with TileContext(nc) as tc:
        with tc.tile_pool(name="xpool", bufs=1) as xpool, \
             tc.tile_pool(name="wpool", bufs=5) as wpool, \
             tc.tile_pool(name="wrpool", bufs=4) as wrpool, \
             tc.tile_pool(name="w2pool", bufs=2) as w2pool, \
             tc.tile_pool(name="spool", bufs=2) as spool, \
             tc.tile_pool(name="opool", bufs=8) as opool, \
             tc.tile_pool(name="ps1", bufs=4, space="PSUM") as ps1, \
             tc.tile_pool(name="ps2", bufs=2, space="PSUM") as ps2:
            for blk in range(NBLK):
                bsl = slice(blk * BBLK, (blk + 1) * BBLK)
                # First group's weights before the bulk x stream so the PE
                # starts as soon as the first x chunk lands.
                w1ts = {0: wpool.tile([P, NIO, P], dt.float8e4,
                                      name=f"w1t_{blk}_0", tag="w1t")}
                nc.sync.dma_start(w1ts[0], w1[0])
                # x main/residual, chunked so matmuls pace the DMA stream.
                # All xm chunks first (consumed first), then rq chunks.
                xmc = []
                rqc = []
                io0 = 0
                for ci, sz in enumerate(XCHS):
                    t = xpool.tile([P, sz, BBLK], dt.float16,
                                   name=f"xm_{blk}_{ci}", tag=f"xm_{ci}")
                    nc.sync.dma_start(t, xm[:, io0:io0 + sz, bsl])
                    xmc.append(t)
                    io0 += sz
                io0 = 0
                for ci, sz in enumerate(XCHS):
                    t = xpool.tile([P, sz, BBLK], dt.float8e4,
                                   name=f"rq_{blk}_{ci}", tag=f"rq_{ci}")
                    nc.sync.dma_start(t, rq[:, io0:io0 + sz, bsl])
                    rqc.append(t)
                    io0 += sz

                # Layer 1: spk1[p(h_inner), ht, b] for this block
                spk1 = spool.tile([P, NHT, BBLK], dt.float8e4, name=f"spk1_{blk}",
                                  tag="spk1")
                # Groups of GB hidden tiles share one Normal->DoubleRow mode
                # switch: all GB main passes (fp16), then all GB residual
                # passes (DR). Mode switches block the LDWEIGHTS pull-ahead
                # (~230ns each), so fewer switches = fewer PE stalls.
                GB = 4
                for gb in range(NHT // GB):
                    hts = list(range(gb * GB, (gb + 1) * GB))
                    psums = {}
                    w1rts = {}
                    for ht in hts:
                        if ht not in w1ts:
                            w1ts[ht] = wpool.tile(
                                [P, NIO, P], dt.float8e4,
                                name=f"w1t_{blk}_{ht}", tag="w1t")
                            nc.sync.dma_start(w1ts[ht], w1[ht])
                        # residual weights derived on-chip: w1r = w1 * 2^-RSH
                        # (exact e4m3 subnormal); saves a second weight stream.
                        w1rts[ht] = wrpool.tile([P, NIO, P], dt.float8e4,
                                                name=f"w1rt_{blk}_{ht}",
                                                tag="w1rt")
                        nc.vector.tensor_scalar(
                            w1rts[ht], w1ts[ht], 2.0 ** -RSH, None,
                            mybir.AluOpType.mult
                        )
                        psums[ht] = ps1.tile([P, BBLK], dt.float32,
                                             name=f"ps1_{blk}_{ht}", tag="ps1")
                    for ht in hts:
                        w1t = w1ts[ht]
                        for io in range(NIO):
                            ci, off = IO2C[io]
                            nc.tensor.matmul(
                                psums[ht], w1t[:, io, :], xmc[ci][:, off, :],
                                start=(io == 0), stop=False,
                            )
                    for ht in hts:
                        w1rt = w1rts[ht]
                        for j in range(NIO // 2):
                            ci, off = IO2C[2 * j]
                            nc.tensor.matmul(
                                psums[ht], w1rt[:, 2 * j:2 * j + 2, :],
                                rqc[ci][:, off:off + 2, :],
                                start=False, stop=(j == NIO // 2 - 1),
                                perf_mode=mybir.MatmulPerfMode.DoubleRow,
                            )
                        nc.vector.tensor_scalar(
                            spk1[:, ht, :], psums[ht], VTH, None,
                            mybir.AluOpType.is_gt
                        )
                        w1ts.pop(ht)

                # Layer 2: out[b, o] for this block (fp8 DoubleRow: hc pairs).
                # og pairs share each spk1 stationary across 2 consecutive
                # matmuls so the 256-col DoubleRow LDWEIGHTS can be deduped /
                # hidden under the other stream.
                for ogp in range(NOG // 2):
                    w2ts = []
                    for half in range(2):
                        og = 2 * ogp + half
                        w2t = w2pool.tile([P, NIO, OGS], dt.float8e4,
                                          name=f"w2t_{blk}_{og}", tag=f"w2t{half}")
                        nc.sync.dma_start(w2t, w2[og])
                        w2ts.append(w2t)
                    for bt in range(NBT):
                        b0 = bt * P
                        psums = [
                            ps2.tile([P, OGS], dt.float32,
                                     name=f"ps2_{blk}_{2 * ogp + half}_{bt}",
                                     tag=f"ps2{half}")
                            for half in range(2)
                        ]
                        for j in range(NIO // 2):
                            lhsT = spk1[:, 2 * j:2 * j + 2, b0:b0 + P]
                            for half in range(2):
                                nc.tensor.matmul(
                                    psums[half],
                                    lhsT,
                                    w2ts[half][:, 2 * j:2 * j + 2, :],
                                    start=(j == 0), stop=(j == NIO // 2 - 1),
                                    perf_mode=mybir.MatmulPerfMode.DoubleRow,
                                )
                        for half in range(2):
                            og = 2 * ogp + half
                            o0 = og * OGS
                            ot = opool.tile([P, OGS], dt.float8e4,
                                            name=f"ot_{blk}_{og}_{bt}", tag="ot")
                            nc.vector.tensor_scalar(
                                ot, psums[half], VTH, None, mybir.AluOpType.is_gt
                            )
                            nc.sync.dma_start(
                                out[blk * BBLK + b0: blk * BBLK + b0 + P,
                                    o0:o0 + OGS], ot
                            )

    nc.finalize()
    return nc


def _get_program():
    if "nc" not in _cache:
        _cache["nc"] = _build_program()
    return _cache["nc"]


def _prep_weights(W1, W2):
    # w1[ht, p, io, h] = sign(W1)[ht*128+h, io*128+p]
    S1 = np.sign(W1).astype(np.float32)
    w1 = np.ascontiguousarray(
        S1.reshape(NHT, P, NIO, P).transpose(0, 3, 2, 1)
    ).astype(F8)
    # w2[og, p, hc, oo] = sign(W2).T[hc*128+p, og*OGS+oo]
    S2T = np.ascontiguousarray(np.sign(W2).astype(np.float32).T)
    w2 = np.ascontiguousarray(
        S2T.reshape(NIO, P, NOG, OGS).transpose(2, 1, 0, 3)
    ).astype(F8)
    return w1, w2


def _tile_x(a):
    # [BC, D] -> [p, io, b]: out[p, io, b] = a[b, io*128+p]
    return np.ascontiguousarray(a.T.reshape(NIO, P, BC).transpose(1, 0, 2))


def _split_x(xs):
    # xs: [BC, D] fp32 -> fp16 main term + e4m3 residual (x512), tiled
    xm = xs.astype(np.float16)
    r = (xs - xm.astype(np.float32)) * RSCALE
    rq = r.astype(F8)
    return _tile_x(xm), _tile_x(rq)


def kernel(x, W1, W2, layer_idx):
    from concourse.bass_utils import run_bass_kernel_spmd

    x = np.asarray(x, dtype=np.float32)
    W1 = np.asarray(W1, dtype=np.float32)
    W2 = np.asarray(W2, dtype=np.float32)

    nc = _get_program()
    w1, w2 = _prep_weights(W1, W2)

    in_maps = []
    for c in range(NCORES):
        xs = x[c * BC:(c + 1) * BC]
        xm, rq = _split_x(xs)
        in_maps.append({"xm": xm, "rq": rq, "w1": w1, "w2": w2})

    res = run_bass_kernel_spmd(nc, in_maps, list(range(NCORES)))
    outs = [res.results[c]["out"].astype(np.float32) for c in range(NCORES)]
    return np.concatenate(outs, axis=0)
